# revision 35
# baseline (speedup 1.0000x reference)
"""Trainium2 Bass kernel for the KB criterion loss.

Math
----
reference:
    diff[b,i,j] = probs[b,j] - probs[b,i]
    loss = sum_ij mean_b (diff^2 * C[i,j]) / (n_pos + 1e-8),  n_pos = count(C > 0)

Expanding the square removes the [B,N,N] intermediate entirely:

    sum_b (P[b,i] - P[b,j])^2 = T_i + T_j - 2*G_ij
        with T_j = sum_b P[b,j]^2   and   G = P^T P  (Gram matrix)

so   total = sum_ij C_ij*T_i + sum_ij C_ij*T_j - 2*sum_ij C_ij*G_ij
           =   A (rows)       +  Bt (cols)      -  2*CG
     loss  = (total / B) / (n_pos + 1e-8)

Sharding (8 cores)
------------------
Shard C by rows: core k owns rows S_k = [128k, 128k+128). P is replicated.
Inputs are column-rolled by 128k so every core runs the same program with
its own row block mapped to local columns [0:128) (so T for the shard rows
is just chunk 0 of the chunked T vector).

The three terms are computed with no cross-engine ladders (v6; each was
measured on HW traces):
  * CG: PE Gram per 512-col PSUM bank (2 bf16 matmuls), DVE multiply by C
    and tensor_scalar-accum row-reduce, pipelined per bank.
  * A = sum_i rowsum_i*T_i: ACT pass over C with the per-partition
    scale AP = (-T_shard/2) straight from PSUM, accum_out per row.
  * Bt = sum_j colsum_j*T_j: both vectors built in PARTITION orientation
    as [128,8] chunk columns via 16 tiny matmuls (contraction over the
    partition dim; chunk k lands in column k), then one tiny DVE
    multiply+reduce. This avoids any [1,N] row-vector op (a 1-partition
    DVE op costs the same as a 128-partition one).
  * n_pos: ACT Sign pass with accum_out.
The per-partition [128,6] partials go straight to HBM; the host does the
final partition/core reduction and the division (the sanctioned scalar
all-reduce).

bf16 inputs (host downcast): halves DMA bytes and avoids multi-pass fp32
matmuls. DMA order P-half0, C, P-half1: the C-gated DVE multiply is the
critical path, not P-half1 (only needed for Gram half 1 / T chunks 4-7).

No PE warmup: the HAM clock gate measurably never opens for this kernel
(bf16 dummy matmuls over 5us of contiguous busy produced zero HAM
transitions), so warmup matmuls only delayed the real work.
"""

import numpy as np
import ml_dtypes

import concourse.bass as bass
import concourse.tile as tile
from concourse import mybir
from concourse.bass_utils import run_bass_kernel_spmd

B = 128
N = 1024
NCORES = 8
SH = N // NCORES  # 128 rows of C per core
NCH = N // SH  # 8 column chunks
F32 = mybir.dt.float32
BF16 = mybir.dt.bfloat16
HALF = 512  # PSUM bank width in fp32


def build_bass() -> bass.Bass:
    nc = bass.Bass()
    p_d = nc.dram_tensor("probs_r", [B, N], BF16, kind="ExternalInput")
    c_d = nc.dram_tensor("co_r", [SH, N], BF16, kind="ExternalInput")
    o_d = nc.dram_tensor("out", [B, 8], F32, kind="ExternalOutput")

    with tile.TileContext(nc) as tc:
        with (
            tc.tile_pool(name="sb", bufs=1) as sb,
            tc.tile_pool(name="ps", bufs=1, space="PSUM") as ps,
        ):
            p_sb = sb.tile([B, N], BF16)
            c0_sb = sb.tile([SH, HALF], BF16)
            c1_sb = sb.tile([SH, HALF], BF16)
            # psq as two tiles: Tile tracks dependencies per TILE, so the
            # th chunk matmuls for half 0 must not inherit a wait on the
            # half-1 square.
            psq_a = sb.tile([B, HALF], BF16)
            psq_b = sb.tile([B, HALF], BF16)
            nh_col = sb.tile([B, 1], BF16)
            ones_col = sb.tile([B, 1], BF16)
            scr_mul = sb.tile([SH, N], BF16)
            scr_red = sb.tile([SH, N], BF16)
            scr_cnt = sb.tile([SH, N], BF16)
            scr_a = sb.tile([SH, N], BF16)
            th_sb = sb.tile([B, NCH], F32)
            scr_bt = sb.tile([B, NCH], F32)
            scr_bt2 = sb.tile([B, NCH], F32)
            partials = sb.tile([B, 8], F32)

            # Gram as two per-bank tiles: with one [B,N] tile, the bank-0
            # C*G multiply inherits a wait on the LAST writer of the whole
            # tile — Gram1, which the scheduler places at the very end of
            # the PE queue (measured ~1.5us of dead DVE time).
            d_ps0 = ps.tile([B, HALF], F32)
            d_ps1 = ps.tile([B, HALF], F32)
            # th chunk 0 gets its own bank: the ACT A-term pass reads it as
            # a scale AP while the PE is still writing the other chunks —
            # separate banks keep that off the PSUM-collision/serialization
            # path.
            th0_ps = ps.tile([B, 1], F32)
            cols_ps = ps.tile([B, 2 * NCH - 1], F32)  # th 1-7, cs 0-7

            # Constants (DVE-born; matmul operands pair per upstream engine)
            nc.vector.memset(nh_col, -0.5)
            nc.vector.memset(ones_col, 1.0)

            # Loads. P half0 first (heads every chain), then C (gates the
            # critical DVE multiply), then P half1 (only Gram1/th4-7).
            nc.sync.dma_start(out=p_sb[:, 0:HALF], in_=p_d[:, 0:HALF])
            nc.sync.dma_start(out=c0_sb, in_=c_d[:, 0:HALF])
            nc.sync.dma_start(out=p_sb[:, HALF:N], in_=p_d[:, HALF:N])
            nc.sync.dma_start(out=c1_sb, in_=c_d[:, HALF:N])

            # Psq = P*P per half
            nc.vector.tensor_mul(psq_a, p_sb[:, 0:HALF], p_sb[:, 0:HALF])
            nc.vector.tensor_mul(psq_b, p_sb[:, HALF:N], p_sb[:, HALF:N])

            # PE program. Gram halves head it; the 16 tiny chunk matmuls
            # build th = -T/2 and cs = colsum in partition orientation.
            nc.tensor.matmul(
                d_ps0, p_sb[:, 0:SH], p_sb[:, 0:HALF],
                start=True, stop=True,
            )
            nc.tensor.matmul(th0_ps, psq_a[:, 0:SH], nh_col, start=True, stop=True)
            for k in range(1, 4):
                nc.tensor.matmul(
                    cols_ps[:, k - 1 : k], psq_a[:, SH * k : SH * (k + 1)], nh_col,
                    start=True, stop=True,
                )
            nc.tensor.matmul(
                d_ps1, p_sb[:, 0:SH], p_sb[:, HALF:N],
                start=True, stop=True,
            )
            for k in range(4, NCH):
                nc.tensor.matmul(
                    cols_ps[:, k - 1 : k],
                    psq_b[:, SH * (k - 4) : SH * (k - 3)], nh_col,
                    start=True, stop=True,
                )
            for k in range(NCH):
                cshalf = c0_sb if k < 4 else c1_sb
                nc.tensor.matmul(
                    cols_ps[:, NCH - 1 + k : NCH + k],
                    cshalf[:, SH * (k % 4) : SH * (k % 4 + 1)], ones_col,
                    start=True, stop=True,
                )

            # DVE: stage -T_shard/2 to SBUF early — the ACT A-term scale AP
            # must be SBUF-resident.
            nc.vector.tensor_copy(th_sb[:, 0:1], th0_ps)

            # DVE: CG per bank (multiply then cheap accum-reduce)
            for h, dps, csb in ((0, d_ps0, c0_sb), (1, d_ps1, c1_sb)):
                js = slice(HALF * h, HALF * (h + 1))
                nc.vector.tensor_mul(scr_mul[:, js], csb, dps)
                nc.vector.tensor_scalar(
                    scr_red[:, js], scr_mul[:, js], 1.0, None,
                    mybir.AluOpType.mult, mybir.AluOpType.add,
                    accum_out=partials[:, h : h + 1],
                )

            # ACT: n_pos and the A-term (scale AP = -T_shard/2), per C
            # half so neither pass waits for the later C DMA
            nc.scalar.activation(
                scr_cnt[:, 0:HALF], c0_sb, mybir.ActivationFunctionType.Sign,
                accum_out=partials[:, 2:3],
            )
            nc.scalar.activation(
                scr_a[:, 0:HALF], c0_sb, mybir.ActivationFunctionType.Copy,
                scale=th_sb[:, 0:1], accum_out=partials[:, 3:4],
            )
            nc.scalar.activation(
                scr_cnt[:, HALF:N], c1_sb, mybir.ActivationFunctionType.Sign,
                accum_out=partials[:, 5:6],
            )
            nc.scalar.activation(
                scr_a[:, HALF:N], c1_sb, mybir.ActivationFunctionType.Copy,
                scale=th_sb[:, 0:1], accum_out=partials[:, 6:7],
            )

            # DVE: Bt = sum over [128,8] of th * cs
            nc.vector.tensor_copy(th_sb[:, 1:NCH], cols_ps[:, 0 : NCH - 1])
            nc.vector.tensor_mul(
                scr_bt, th_sb, cols_ps[:, NCH - 1 : 2 * NCH - 1]
            )
            nc.vector.tensor_scalar(
                scr_bt2, scr_bt, 1.0, None,
                mybir.AluOpType.mult, mybir.AluOpType.add,
                accum_out=partials[:, 4:5],
            )

            # ship the [128,6] per-partition partials; host reduces.
            nc.sync.dma_start(out=o_d[:, :], in_=partials)

    _split_multi_waits(nc)
    return nc


def _split_multi_waits(nc: bass.Bass):
    """This walrus build accepts only ONE sync-wait per instruction
    ("Too many sync wait commands"). Tile's kernel-tail drain carries one
    wait per live semaphore; peel the extras onto same-engine NOPs that
    each stall on a single semaphore — semantically identical."""
    for bb in nc.main_func.blocks:
        insts = bb.instructions
        i = 0
        while i < len(insts):
            ins = insts[i]
            si = getattr(ins, "sync_info", None)
            if si is not None and si.on_wait is not None and len(si.on_wait) > 1:
                waits = list(si.on_wait)
                nops = []
                for j, w in enumerate(waits[:-1]):
                    nop = mybir.InstNoOp(
                        name=f"{ins.name}-wsplit{j}",
                        sync_info=mybir.SyncInfo(on_wait=[w], on_update=[]),
                        bass_nofuse=True,
                        engine=ins.engine,
                    )
                    nc.register_instruction(nop, overwrite=True)
                    nops.append(nop)
                si.on_wait = [waits[-1]]
                insts[i:i] = nops
                i += len(nops)
            i += 1


_NC = None


def _get_nc() -> bass.Bass:
    global _NC
    if _NC is None:
        _NC = build_bass()
    return _NC


def make_in_maps(probs: np.ndarray, co_matrix: np.ndarray):
    probs = np.ascontiguousarray(np.asarray(probs, dtype=np.float32))
    co_matrix = np.ascontiguousarray(np.asarray(co_matrix, dtype=np.float32))
    in_maps = []
    for k in range(NCORES):
        shift = -SH * k
        p_r = np.ascontiguousarray(
            np.roll(probs, shift, axis=1).astype(ml_dtypes.bfloat16)
        )
        c_r = np.ascontiguousarray(
            np.roll(co_matrix[SH * k : SH * (k + 1), :], shift, axis=1).astype(
                ml_dtypes.bfloat16
            )
        )
        in_maps.append({"probs_r": p_r, "co_r": c_r})
    return in_maps


def finish(outs: np.ndarray) -> np.ndarray:
    """outs: [NCORES, 128, 8] per-partition partials:
    col0/1 = sum_j C*G per bank, col2/5 = n_pos per C half,
    col3/6 = -A/2 per C half, col4 = -Bt/2.

    total = A + Bt - 2*CG = -2 * (col0+col1+col3+col4+col6)."""
    o = outs.astype(np.float64)
    total = np.float32(
        -2.0
        * (o[:, :, 0] + o[:, :, 1] + o[:, :, 3] + o[:, :, 4] + o[:, :, 6]).sum()
    )
    npos = np.float32((o[:, :, 2] + o[:, :, 5]).sum())
    loss = (total / np.float32(B)) / (npos + np.float32(1e-8))
    return np.array(loss, dtype=np.float32)


TRACE = False
TRACE_DIR = None
LAST_RESULTS = None


def kernel(probs: np.ndarray, co_matrix: np.ndarray) -> np.ndarray:
    global LAST_RESULTS
    nc = _get_nc()
    in_maps = make_in_maps(probs, co_matrix)
    kwargs = {}
    if TRACE:
        kwargs = dict(trace=True, trace_cores=list(range(NCORES)), tmpdir=TRACE_DIR)
    res = run_bass_kernel_spmd(nc, in_maps, list(range(NCORES)), **kwargs)
    LAST_RESULTS = res
    outs = np.stack([r["out"] for r in res.results])
    return finish(outs)


# revision 36
# speedup vs baseline: 1.0959x; 1.0959x over previous
"""Trainium2 Bass kernel for the KB criterion loss.

Math
----
reference:
    diff[b,i,j] = probs[b,j] - probs[b,i]
    loss = sum_ij mean_b (diff^2 * C[i,j]) / (n_pos + 1e-8),  n_pos = count(C > 0)

Expanding the square removes the [B,N,N] intermediate entirely:

    sum_b (P[b,i] - P[b,j])^2 = T_i + T_j - 2*G_ij
        with T_j = sum_b P[b,j]^2   and   G = P^T P  (Gram matrix)

so   total = sum_ij C_ij*T_i + sum_ij C_ij*T_j - 2*sum_ij C_ij*G_ij
           =   A (rows)       +  Bt (cols)      -  2*CG
     loss  = (total / B) / (n_pos + 1e-8)

Sharding (8 cores)
------------------
Shard C by rows: core k owns rows S_k = [128k, 128k+128). P is replicated.
Inputs are column-rolled by 128k so every core runs the same program with
its own row block mapped to local columns [0:128) (so T for the shard rows
is just chunk 0 of the chunked T vector).

The three terms are computed with no cross-engine ladders (v6; each was
measured on HW traces):
  * CG: PE Gram per 512-col PSUM bank (2 bf16 matmuls), DVE multiply by C
    and tensor_scalar-accum row-reduce, pipelined per bank.
  * A = sum_i rowsum_i*T_i: ACT pass over C with the per-partition
    scale AP = (-T_shard/2) straight from PSUM, accum_out per row.
  * Bt = sum_j colsum_j*T_j: both vectors built in PARTITION orientation
    as [128,8] chunk columns via 16 tiny matmuls (contraction over the
    partition dim; chunk k lands in column k), then one tiny DVE
    multiply+reduce. This avoids any [1,N] row-vector op (a 1-partition
    DVE op costs the same as a 128-partition one).
  * n_pos: ACT Sign pass with accum_out.
The per-partition [128,6] partials go straight to HBM; the host does the
final partition/core reduction and the division (the sanctioned scalar
all-reduce).

bf16 inputs (host downcast): halves DMA bytes and avoids multi-pass fp32
matmuls. DMA order P-half0, C, P-half1: the C-gated DVE multiply is the
critical path, not P-half1 (only needed for Gram half 1 / T chunks 4-7).

No PE warmup: the HAM clock gate measurably never opens for this kernel
(bf16 dummy matmuls over 5us of contiguous busy produced zero HAM
transitions), so warmup matmuls only delayed the real work.
"""

import numpy as np
import ml_dtypes

import concourse.bass as bass
import concourse.tile as tile
from concourse import mybir
from concourse.bass_utils import run_bass_kernel_spmd

B = 128
N = 1024
NCORES = 8
SH = N // NCORES  # 128 rows of C per core
NCH = N // SH  # 8 column chunks
F32 = mybir.dt.float32
BF16 = mybir.dt.bfloat16
HALF = 512  # PSUM bank width in fp32


def build_bass() -> bass.Bass:
    nc = bass.Bass()
    p_d = nc.dram_tensor("probs_r", [B, N], BF16, kind="ExternalInput")
    c_d = nc.dram_tensor("co_r", [SH, N], BF16, kind="ExternalInput")
    o_d = nc.dram_tensor("out", [B, 6], F32, kind="ExternalOutput")

    with tile.TileContext(nc) as tc:
        with (
            tc.tile_pool(name="sb", bufs=1) as sb,
            tc.tile_pool(name="ps", bufs=1, space="PSUM") as ps,
        ):
            p_sb = sb.tile([B, N], BF16)
            c_sb = sb.tile([SH, N], BF16)
            # psq as two tiles: Tile tracks dependencies per TILE, so the
            # th chunk matmuls for half 0 must not inherit a wait on the
            # half-1 square.
            psq_a = sb.tile([B, HALF], BF16)
            psq_b = sb.tile([B, HALF], BF16)
            nh_col = sb.tile([B, 1], BF16)
            ones_col = sb.tile([B, 1], BF16)
            scr_mul = sb.tile([SH, N], BF16)
            scr_red = sb.tile([SH, N], BF16)
            scr_cnt = sb.tile([SH, N], BF16)
            scr_a = sb.tile([SH, N], BF16)
            th_sb = sb.tile([B, NCH], F32)
            scr_bt = sb.tile([B, NCH], F32)
            scr_bt2 = sb.tile([B, NCH], F32)
            partials = sb.tile([B, 6], F32)

            # Gram as two per-bank tiles: with one [B,N] tile, the bank-0
            # C*G multiply inherits a wait on the LAST writer of the whole
            # tile — Gram1, which the scheduler places at the very end of
            # the PE queue (measured ~1.5us of dead DVE time).
            d_ps0 = ps.tile([B, HALF], F32)
            d_ps1 = ps.tile([B, HALF], F32)
            # th chunk 0 gets its own bank: the ACT A-term pass reads it as
            # a scale AP while the PE is still writing the other chunks —
            # separate banks keep that off the PSUM-collision/serialization
            # path.
            th0_ps = ps.tile([B, 1], F32)
            cols_ps = ps.tile([B, 2 * NCH - 1], F32)  # th 1-7, cs 0-7

            # Constants (DVE-born; matmul operands pair per upstream engine)
            nc.vector.memset(nh_col, -0.5)
            nc.vector.memset(ones_col, 1.0)

            # Loads. P half0 first (heads every chain), then C (gates the
            # critical DVE multiply), then P half1 (only Gram1/th4-7).
            nc.sync.dma_start(out=p_sb[:, 0:HALF], in_=p_d[:, 0:HALF])
            nc.sync.dma_start(out=c_sb, in_=c_d[:, :])
            nc.sync.dma_start(out=p_sb[:, HALF:N], in_=p_d[:, HALF:N])

            # Psq = P*P per half
            nc.vector.tensor_mul(psq_a, p_sb[:, 0:HALF], p_sb[:, 0:HALF])
            nc.vector.tensor_mul(psq_b, p_sb[:, HALF:N], p_sb[:, HALF:N])

            # PE program. Gram halves head it; the 16 tiny chunk matmuls
            # build th = -T/2 and cs = colsum in partition orientation.
            nc.tensor.matmul(
                d_ps0, p_sb[:, 0:SH], p_sb[:, 0:HALF],
                start=True, stop=True,
            )
            nc.tensor.matmul(th0_ps, psq_a[:, 0:SH], nh_col, start=True, stop=True)
            for k in range(1, 4):
                nc.tensor.matmul(
                    cols_ps[:, k - 1 : k], psq_a[:, SH * k : SH * (k + 1)], nh_col,
                    start=True, stop=True,
                )
            nc.tensor.matmul(
                d_ps1, p_sb[:, 0:SH], p_sb[:, HALF:N],
                start=True, stop=True,
            )
            for k in range(4, NCH):
                nc.tensor.matmul(
                    cols_ps[:, k - 1 : k],
                    psq_b[:, SH * (k - 4) : SH * (k - 3)], nh_col,
                    start=True, stop=True,
                )
            for k in range(NCH):
                nc.tensor.matmul(
                    cols_ps[:, NCH - 1 + k : NCH + k],
                    c_sb[:, SH * k : SH * (k + 1)], ones_col,
                    start=True, stop=True,
                )

            # DVE: stage -T_shard/2 to SBUF early — the ACT A-term scale AP
            # must be SBUF-resident.
            nc.vector.tensor_copy(th_sb[:, 0:1], th0_ps)

            # DVE: CG per bank (multiply then cheap accum-reduce)
            for h, dps in ((0, d_ps0), (1, d_ps1)):
                js = slice(HALF * h, HALF * (h + 1))
                nc.vector.tensor_mul(scr_mul[:, js], c_sb[:, js], dps)
                nc.vector.tensor_scalar(
                    scr_red[:, js], scr_mul[:, js], 1.0, None,
                    mybir.AluOpType.mult, mybir.AluOpType.add,
                    accum_out=partials[:, h : h + 1],
                )

            # ACT: n_pos and the A-term (scale AP = -T_shard/2 from PSUM)
            nc.scalar.activation(
                scr_cnt, c_sb, mybir.ActivationFunctionType.Sign,
                accum_out=partials[:, 2:3],
            )
            nc.scalar.activation(
                scr_a, c_sb, mybir.ActivationFunctionType.Copy,
                scale=th_sb[:, 0:1], accum_out=partials[:, 3:4],
            )

            # DVE: Bt = sum over [128,8] of th * cs
            nc.vector.tensor_copy(th_sb[:, 1:NCH], cols_ps[:, 0 : NCH - 1])
            nc.vector.tensor_mul(
                scr_bt, th_sb, cols_ps[:, NCH - 1 : 2 * NCH - 1]
            )
            nc.vector.tensor_scalar(
                scr_bt2, scr_bt, 1.0, None,
                mybir.AluOpType.mult, mybir.AluOpType.add,
                accum_out=partials[:, 4:5],
            )

            # ship the [128,6] per-partition partials; host reduces.
            nc.sync.dma_start(out=o_d[:, :], in_=partials)

    _split_multi_waits(nc)
    return nc


def _split_multi_waits(nc: bass.Bass):
    """This walrus build accepts only ONE sync-wait per instruction
    ("Too many sync wait commands"). Tile's kernel-tail drain carries one
    wait per live semaphore; peel the extras onto same-engine NOPs that
    each stall on a single semaphore — semantically identical."""
    for bb in nc.main_func.blocks:
        insts = bb.instructions
        i = 0
        while i < len(insts):
            ins = insts[i]
            si = getattr(ins, "sync_info", None)
            if si is not None and si.on_wait is not None and len(si.on_wait) > 1:
                waits = list(si.on_wait)
                nops = []
                for j, w in enumerate(waits[:-1]):
                    nop = mybir.InstNoOp(
                        name=f"{ins.name}-wsplit{j}",
                        sync_info=mybir.SyncInfo(on_wait=[w], on_update=[]),
                        bass_nofuse=True,
                        engine=ins.engine,
                    )
                    nc.register_instruction(nop, overwrite=True)
                    nops.append(nop)
                si.on_wait = [waits[-1]]
                insts[i:i] = nops
                i += len(nops)
            i += 1


_NC = None


def _get_nc() -> bass.Bass:
    global _NC
    if _NC is None:
        _NC = build_bass()
    return _NC


def make_in_maps(probs: np.ndarray, co_matrix: np.ndarray):
    probs = np.ascontiguousarray(np.asarray(probs, dtype=np.float32))
    co_matrix = np.ascontiguousarray(np.asarray(co_matrix, dtype=np.float32))
    in_maps = []
    for k in range(NCORES):
        shift = -SH * k
        p_r = np.ascontiguousarray(
            np.roll(probs, shift, axis=1).astype(ml_dtypes.bfloat16)
        )
        c_r = np.ascontiguousarray(
            np.roll(co_matrix[SH * k : SH * (k + 1), :], shift, axis=1).astype(
                ml_dtypes.bfloat16
            )
        )
        in_maps.append({"probs_r": p_r, "co_r": c_r})
    return in_maps


def finish(outs: np.ndarray) -> np.ndarray:
    """outs: [NCORES, 128, 6] per-partition partials:
    col0/1 = sum_j C*G per bank, col2 = n_pos, col3 = -A/2, col4 = -Bt/2.

    total = A + Bt - 2*CG = -2 * (col3 + col4 + col0 + col1)."""
    o = outs.astype(np.float64)
    total = np.float32(
        -2.0 * (o[:, :, 0] + o[:, :, 1] + o[:, :, 3] + o[:, :, 4]).sum()
    )
    npos = np.float32(o[:, :, 2].sum())
    loss = (total / np.float32(B)) / (npos + np.float32(1e-8))
    return np.array(loss, dtype=np.float32)


TRACE = False
TRACE_DIR = None
LAST_RESULTS = None


def kernel(probs: np.ndarray, co_matrix: np.ndarray) -> np.ndarray:
    global LAST_RESULTS
    nc = _get_nc()
    in_maps = make_in_maps(probs, co_matrix)
    kwargs = {}
    if TRACE:
        kwargs = dict(trace=True, trace_cores=list(range(NCORES)), tmpdir=TRACE_DIR)
    res = run_bass_kernel_spmd(nc, in_maps, list(range(NCORES)), **kwargs)
    LAST_RESULTS = res
    outs = np.stack([r["out"] for r in res.results])
    return finish(outs)
